# revision 24
# baseline (speedup 1.0000x reference)
"""CBAM block (channel + spatial attention) Trainium2 Bass kernel, v3.

Problem: x [32, 56, 56, 256] f32; data-parallel over batch across 8 NeuronCores
(4 images per core).  Everything is hardcoded for these shapes.

Per-core dataflow (B=4 images, each [3136(hw), 256(c)]), bf16 on chip:

  Layout: flat row n of an image maps to (partition p, block t) as
    group A: p in [0, 64),   t in [0, 25): n = 25*p + t
    group B: p in [64, 128), t in [0, 24): n = 1600 + 24*(p-64) + t

  Input: f32 on the two HWDGE rings (sync=group A, scalar=group B), then one
  big 3D ACT copy per image casts to bf16 (Xb); every later DVE scan runs at
  the 2x_1P bf16 rate.

  Phase A (channel attention): pairwise max tree over t (6 DVE ops) -> 2 PE
  transposes -> DVE max -> statsT col 1; sum over hw via 13 pair-accumulated
  PE matmuls (rhs [128, 512] bf16) -> psum row -> fold + transposes ->
  statsT col 0; tiny f32 MLP; ca broadcast by ones-matmul -> bca bf16.

  Phase B1: XR = Xb * bca as ONE 3D DVE op (in1 broadcast over t via
  stride-0 AP, still 2x).  sum_c / max_c as bf16 pairwise trees to width 16
  plus one small reduce -> maps bf16.

  Phase B2 (spatial 7x7 conv): Toeplitz bands built ONCE from an inline 0/1
  diagonal-mask constant: tmp98[(ch,dw,dh), (a,b)] = dmask * w98 (one DVE
  tensor_scalar), 7 selector matmuls fold dh -> tband [(ch,dw), (a,b)],
  DRAM round trip re-lays it as wband112 [(ch,h_in)=112, dw, h_out].
  Per image: maps -> flat DRAM -> cin2 [(ch,h_in), w] (6 small gpsimd DMAs),
  7 accumulated matmuls (lhsT = wband112[:, dw, :], w-shift via column
  windows) -> psum [56, 56], ACT sigmoid evac, 2 SBUF->SBUF DMAs -> saf.

  Phase B3: XR *= saf[p, t]: a few blocks on DVE tensor_scalar, the rest on
  ACT activation-with-scale; 2 bf16 out-DMAs; host casts to f32.
"""

import os

import numpy as np
import ml_dtypes

import concourse.bass as bass
import concourse.bacc as bacc
import concourse.tile as tile
from concourse import mybir
from concourse.bass_types import AP
from concourse.bass_utils import run_bass_kernel_spmd

F32 = mybir.dt.float32
BF16 = mybir.dt.bfloat16
AX = mybir.AxisListType
OP = mybir.AluOpType
ACT = mybir.ActivationFunctionType

P = 128          # partitions
NB = 25          # blocks in group A (group B has 24)
NBF = 24         # full-width blocks
HALF = 64        # partitions in group A / valid rows in block 24
C = 256          # channels
HW = 3136        # 56*56
GA = 1600        # rows in group A (64 * 25)
NIMG = 4         # images per core
NCORES = 8
NDVE_B3 = 14      # leading blocks of B3 applied on DVE; rest on ACT

_CACHE: dict = {}


def _build_nc() -> bass.Bass:
    nc = bacc.Bacc()

    x_d = nc.dram_tensor("x", [NIMG, 56, 56, C], F32, kind="ExternalInput")
    w1_d = nc.dram_tensor("w1", [C, 16], F32, kind="ExternalInput")
    b1_d = nc.dram_tensor("b1", [16], F32, kind="ExternalInput")
    w2_d = nc.dram_tensor("w2", [16, C], F32, kind="ExternalInput")
    b2_d = nc.dram_tensor("b2", [C], F32, kind="ExternalInput")
    cw_d = nc.dram_tensor("conv_w", [7, 7, 2, 1], F32, kind="ExternalInput")
    out_d = nc.dram_tensor("out", [NIMG, 56, 56, C], BF16, kind="ExternalOutput")

    ident_d = nc.inline_tensor(np.eye(128, dtype=np.float32), name="ident128")

    # dmask98[(ch,dh,dw), (a, b)] = 1 iff a - b == dh - 3  (bands over h)
    dm = np.zeros((7, 56, 56), dtype=ml_dtypes.bfloat16)
    for dh in range(7):
        for a in range(56):
            b = a - (dh - 3)
            if 0 <= b < 56:
                dm[dh, a, b] = 1.0
    dmask98_np = np.broadcast_to(
        dm[None, :, None, :, :], (2, 7, 7, 56, 56)
    ).reshape(98, HW)
    dmask98_d = nc.inline_tensor(np.ascontiguousarray(dmask98_np), name="dmask98")

    # sel98[(ch,dh,dw), (ch',dw')] = 1 iff (ch,dw) == (ch',dw')
    sel_np = np.zeros((98, 14), dtype=ml_dtypes.bfloat16)
    for ch in range(2):
        for dh in range(7):
            for dw in range(7):
                sel_np[ch * 49 + dh * 7 + dw, ch * 7 + dw] = 1.0
    sel98_d = nc.inline_tensor(sel_np, name="sel98")

    wscale_np = np.ones((98, 1), dtype=np.float32)
    wscale_np[0:49] = 1.0 / C  # fold the mean 1/C into the ch0 conv taps
    wscale_d = nc.inline_tensor(wscale_np, name="wscale")

    x_hwc = x_d[:].rearrange("b h w c -> b (h w) c")
    out_hwc = out_d[:].rearrange("b h w c -> b (h w) c")

    with tile.TileContext(nc) as tc:
        import contextlib

        with contextlib.ExitStack() as ctx:
            cpool = ctx.enter_context(tc.tile_pool(name="cpool", bufs=1))
            xpool = ctx.enter_context(tc.tile_pool(name="xpool", bufs=1))
            work = ctx.enter_context(tc.tile_pool(name="work", bufs=2))
            small = ctx.enter_context(tc.tile_pool(name="small", bufs=3))
            psS = ctx.enter_context(tc.tile_pool(name="psS", bufs=2, space="PSUM"))
            psM = ctx.enter_context(tc.tile_pool(name="psM", bufs=3, space="PSUM"))
            psC = ctx.enter_context(tc.tile_pool(name="psC", bufs=2, space="PSUM"))
            dpool = ctx.enter_context(tc.tile_pool(name="dpool", bufs=2, space="DRAM"))

            # ---------------- constants & weights ----------------
            identf = cpool.tile([128, 128], F32)
            nc.gpsimd.dma_start(out=identf, in_=ident_d[:])
            identb = cpool.tile([128, 128], BF16)
            nc.vector.tensor_copy(out=identb, in_=identf)

            w1_sb = cpool.tile([128, 2, 16], F32)
            nc.gpsimd.dma_start(out=w1_sb, in_=w1_d[:].rearrange("(j p) m -> p j m", p=128))
            w2_sb = cpool.tile([16, 2, 128], F32)
            nc.gpsimd.dma_start(out=w2_sb, in_=w2_d[:].rearrange("k (j m) -> k j m", j=2))
            b1_sb = cpool.tile([16, 1], F32)
            nc.gpsimd.dma_start(out=b1_sb, in_=b1_d[:].rearrange("(p o) -> p o", o=1))
            b2_sb = cpool.tile([128, 2], F32)
            nc.gpsimd.dma_start(out=b2_sb, in_=b2_d[:].rearrange("(j p) -> p j", p=128))
            b2x2 = cpool.tile([128, 2], F32)

            # conv weights in (ch, dh, dw) partition order
            w98f = cpool.tile([98, 1], F32)
            nc.gpsimd.dma_start(out=w98f, in_=cw_d[:].transpose([2, 0, 1, 3]))

            ones_col = cpool.tile([128, 1], BF16)
            nc.vector.memset(ones_col, 1.0)
            ones_rb = cpool.tile([1, 128], BF16)
            nc.vector.memset(ones_rb, 1.0)

            # ---------------- one-time Toeplitz band build ------------------
            # (issued AFTER the input loads so its ACT evac copies don't sit
            # ahead of the group-B load DMAs in the scalar engine FIFO)
            sel_sb = cpool.tile([98, 14], BF16)
            nc.gpsimd.dma_start(out=sel_sb, in_=sel98_d[:])
            wband = cpool.tile([112, 7, 56], BF16)

            wscale = cpool.tile([98, 1], F32)
            nc.gpsimd.dma_start(out=wscale, in_=wscale_d[:])

            def build_bands():
                nc.scalar.activation(out=b2x2, in_=b2_sb, func=ACT.Copy, scale=2.0)
                nc.vector.tensor_mul(out=w98f, in0=w98f, in1=wscale)
                dmask_sb = work.tile([98, HW], BF16, tag="s1", bufs=1)
                nc.gpsimd.dma_start(out=dmask_sb, in_=dmask98_d[:])
                tmp98 = work.tile([98, HW], BF16, tag="m12", bufs=1)
                nc.vector.tensor_scalar_mul(out=tmp98, in0=dmask_sb, scalar1=w98f)

                tband = work.tile([14, HW], BF16, tag="s1x", bufs=1)
                for c7 in range(7):
                    ptb = psC.tile([128, 448], F32, tag="pconv", name=f"ptb{c7}")
                    nc.tensor.matmul(
                        ptb[0:14, :],
                        lhsT=sel_sb,
                        rhs=tmp98[:, 448 * c7 : 448 * (c7 + 1)],
                        start=True,
                        stop=True,
                    )
                    nc.scalar.copy(
                        out=tband[:, 448 * c7 : 448 * (c7 + 1)], in_=ptb[0:14, :]
                    )
                tbd = dpool.tile([14, HW], BF16, tag="tbd", bufs=1)
                nc.gpsimd.dma_start(out=tbd[:], in_=tband)
                tb = tbd[:]
                for ch in range(2):
                    in_ap = AP(
                        tb.tensor,
                        tb.offset + ch * 7 * HW,
                        [[56, 56], [HW, 7], [1, 56]],
                    )
                    nc.gpsimd.dma_start(
                        out=wband[56 * ch : 56 * ch + 56, :, :], in_=in_ap
                    )

            # ---------------- big SBUF state ----------------
            Xb = xpool.tile([P, NIMG, NB, C], BF16)
            XR = Xb  # B1/B3 run in place; Xb is dead after phase A reads it
            xf_tiles = {}
            bca = cpool.tile([P, NIMG, C], BF16)
            maps = cpool.tile([P, NIMG, 2, NB], BF16)
            saf = cpool.tile([P, NIMG, NB], F32)
            safb = cpool.tile([P, NIMG, NB], BF16)
            statsT = cpool.tile([128, NIMG, 2, 2], F32)
            caT = cpool.tile([128, NIMG, 2], F32)

            # PE warm-up matmuls touching constant lhsT sources
            pwu = psM.tile([128, 4], F32, tag="mlp")
            nc.tensor.matmul(
                pwu[0:4, 0:4], lhsT=identb[:, 0:4], rhs=identb[:, 0:4],
                start=True, stop=True,
            )
            nc.tensor.matmul(
                pwu[0:4, 0:4], lhsT=identf[:, 0:4], rhs=identf[:, 0:4],
                start=True, stop=True,
            )
            nc.tensor.matmul(
                pwu[0:1, 0:4], lhsT=ones_col[0:1, :], rhs=ones_rb[:, 0:4],
                start=True, stop=True,
            )
            nc.tensor.matmul(
                pwu[0:4, 0:4], lhsT=w1_sb[:, 0, 0:4], rhs=w1_sb[:, 0, 0:4],
                start=True, stop=True,
            )
            nc.tensor.matmul(
                pwu[0:4, 0:4], lhsT=w2_sb[:, 0, 0:4], rhs=w2_sb[:, 0, 0:4],
                start=True, stop=True,
            )
            nc.tensor.matmul(
                pwu[0:4, 0:4], lhsT=wband[0:4, 0, 0:4], rhs=wband[0:4, 0, 0:4],
                start=True, stop=True,
            )

            # ---------------- DMA in (f32, both HWDGE rings) ----------------
            def load(b):
                if b in (0, 3):
                    nc.gpsimd.dma_start(
                        out=Xb[0:P, b, 0:NBF, :],
                        in_=x_hwc[b, 0:3072, :].rearrange("(p t) c -> p t c", t=NBF),
                    )
                    nc.gpsimd.dma_start(
                        out=Xb[0:HALF, b, NBF:NB, :],
                        in_=x_hwc[b, 3072:HW, :].rearrange("(p o) c -> p o c", o=1),
                    )
                    return
                xf = work.tile([P, NB, C], F32, tag="xf", name=f"xf{b}", bufs=2)
                xf_tiles[b] = xf
                ring = nc.sync if b == 1 else nc.scalar
                ring.dma_start(
                    out=xf[0:P, 0:NBF, :],
                    in_=x_hwc[b, 0:3072, :].rearrange("(p t) c -> p t c", t=NBF),
                )
                ring.dma_start(
                    out=xf[0:HALF, NBF:NB, :],
                    in_=x_hwc[b, 3072:HW, :].rearrange("(p o) c -> p o c", o=1),
                )

            def cast(b):
                if b in (1, 2):
                    nc.scalar.copy(out=Xb[:, b], in_=xf_tiles[b][:])

            # ---------------- phase A (channel attention) -------------------
            def phase_a(b):
                m12 = work.tile([P, 12, C], BF16, tag="m12", bufs=1)
                m6 = work.tile([P, 6, C], BF16, tag="m6", bufs=1)
                m3 = work.tile([P, 3, C], BF16, tag="m3", bufs=1)
                acc = work.tile([P, C], BF16, tag="acc")
                nc.vector.tensor_max(
                    out=m12, in0=Xb[:, b, 0:12, :], in1=Xb[:, b, 12:NBF, :]
                )
                nc.vector.tensor_max(out=m6, in0=m12[:, 0:6, :], in1=m12[:, 6:12, :])
                nc.vector.tensor_max(out=m3, in0=m6[:, 0:3, :], in1=m6[:, 3:6, :])
                nc.vector.tensor_max(out=acc, in0=m3[:, 0, :], in1=m3[:, 1, :])
                nc.vector.tensor_max(out=acc, in0=acc, in1=m3[:, 2, :])
                nc.vector.tensor_max(
                    out=acc[0:HALF], in0=acc[0:HALF], in1=Xb[0:HALF, b, NBF, :]
                )
                pmaxT = psM.tile([128, 2, 128], BF16, tag="mlp")
                for j in range(2):
                    nc.tensor.transpose(
                        pmaxT[:, j, :], acc[:, j * 128 : (j + 1) * 128], identb
                    )
                nc.vector.tensor_reduce(
                    out=statsT[:, b, :, 1:2], in_=pmaxT, axis=AX.X, op=OP.max
                )

                # sum over hw: 13 pair-accumulated matmuls -> [1, 512] psum
                ps = psS.tile([1, 512], F32, tag="ps")
                for k in range(12):
                    nc.tensor.matmul(
                        ps,
                        lhsT=ones_col,
                        rhs=Xb[:, b, 2 * k : 2 * k + 2, :],
                        start=(k == 0),
                        stop=False,
                    )
                nc.tensor.matmul(
                    ps[:, 0:C],
                    lhsT=ones_col[0:HALF],
                    rhs=Xb[0:HALF, b, NBF, :],
                    start=False,
                    stop=True,
                )
                ssb = small.tile([1, 512], F32, tag="ssb")
                nc.vector.tensor_copy(out=ssb, in_=ps)
                srow = small.tile([1, C], F32, tag="srow")
                nc.vector.tensor_add(out=srow, in0=ssb[:, 0:C], in1=ssb[:, C:512])
                pavgT = psM.tile([128, 2, 1], F32, tag="mlp")
                for j in range(2):
                    nc.tensor.transpose(
                        pavgT[:, j, :],
                        srow[:, j * 128 : (j + 1) * 128],
                        identf[0:1, 0:1],
                    )
                nc.scalar.activation(
                    out=statsT[:, b, :, 0:1], in_=pavgT, func=ACT.Copy, scale=1.0 / HW
                )

                # MLP layer 1: h = relu(W1^T statsT + b1)
                ph = psM.tile([16, 2], F32, tag="mlp")
                for j in range(2):
                    nc.tensor.matmul(
                        ph,
                        lhsT=w1_sb[:, j, :],
                        rhs=statsT[:, b, j, :],
                        start=(j == 0),
                        stop=(j == 1),
                    )
                h_sb = small.tile([16, 2], F32, tag="h_sb")
                nc.scalar.activation(
                    out=h_sb, in_=ph, func=ACT.Relu, bias=b1_sb, scale=1.0
                )

                # layer 2 + combine + sigmoid -> caT [128, 2]
                for j in range(2):
                    pc = psM.tile([128, 2], F32, tag="mlp")
                    nc.tensor.matmul(
                        pc, lhsT=w2_sb[:, j, :], rhs=h_sb, start=True, stop=True
                    )
                    pc_sb = small.tile([128, 2], F32, tag="pc_sb")
                    nc.vector.tensor_copy(out=pc_sb, in_=pc)
                    catmp = small.tile([128, 1], F32, tag="catmp")
                    nc.vector.tensor_add(
                        out=catmp, in0=pc_sb[:, 0:1], in1=pc_sb[:, 1:2]
                    )
                    nc.scalar.activation(
                        out=caT[:, b, j : j + 1],
                        in_=catmp,
                        func=ACT.Sigmoid,
                        bias=b2x2[:, j : j + 1],
                        scale=1.0,
                    )

                # broadcast ca over partitions: bca[:, b, :] (bf16)
                pcr = psM.tile([1, 2, 128], F32, tag="mlp")
                for j in range(2):
                    nc.tensor.transpose(pcr[:, j, :], caT[:, b, j : j + 1], identf)
                ca_row = small.tile([1, C], BF16, tag="ca_row")
                nc.vector.tensor_copy(
                    out=ca_row, in_=pcr.rearrange("p j m -> p (j m)")
                )
                pbca = psS.tile([P, C], F32, tag="ps")
                nc.tensor.matmul(pbca, lhsT=ones_rb, rhs=ca_row, start=True, stop=True)
                nc.scalar.copy(out=bca[:, b, :], in_=pbca)

            # ------------- phase B1: XR = Xb * ca; spatial maps -------------
            def phase_b1(b):
                nc.vector.tensor_mul(
                    out=XR[:, b],
                    in0=Xb[:, b],
                    in1=bca[:, b, :][:, None, :].broadcast_to([P, NB, C]),
                )
                s1 = work.tile([P, NB, 128], BF16, tag="s1", bufs=1)
                s2 = work.tile([P, NB, 64], BF16, tag="s2", bufs=1)
                s3 = work.tile([P, NB, 32], BF16, tag="s3", bufs=1)
                s4 = work.tile([P, NB, 16], BF16, tag="s4", bufs=1)
                nc.vector.tensor_add(
                    out=s1, in0=XR[:, b, :, 0:128], in1=XR[:, b, :, 128:C]
                )
                nc.vector.tensor_add(out=s2, in0=s1[:, :, 0:64], in1=s1[:, :, 64:128])
                nc.vector.tensor_add(out=s3, in0=s2[:, :, 0:32], in1=s2[:, :, 32:64])
                nc.vector.tensor_add(out=s4, in0=s3[:, :, 0:16], in1=s3[:, :, 16:32])
                with nc.allow_low_precision("mean map tolerates bf16"):
                    nc.vector.tensor_reduce(
                        out=maps[:, b, 0, :], in_=s4, axis=AX.X, op=OP.add
                    )
                nc.vector.tensor_max(
                    out=s1, in0=XR[:, b, :, 0:128], in1=XR[:, b, :, 128:C]
                )
                nc.vector.tensor_max(out=s2, in0=s1[:, :, 0:64], in1=s1[:, :, 64:128])
                nc.vector.tensor_max(out=s3, in0=s2[:, :, 0:32], in1=s2[:, :, 32:64])
                nc.vector.tensor_max(out=s4, in0=s3[:, :, 0:16], in1=s3[:, :, 16:32])
                nc.vector.tensor_reduce(
                    out=maps[:, b, 1, :], in_=s4, axis=AX.X, op=OP.max
                )

            # ------------- phase B2: 7x7 conv via Toeplitz bands ------------
            def phase_b2(b):
                mdr = dpool.tile([2, HW], BF16, tag="mdr")
                for ch in range(2):
                    nc.sync.dma_start(
                        out=mdr[ch, 0:3072].rearrange("(p t) -> p t", t=NBF),
                        in_=maps[0:P, b, ch, 0:NBF],
                    )
                    nc.sync.dma_start(
                        out=mdr[ch, 3072:HW].rearrange("(p o) -> p o", o=1),
                        in_=maps[0:HALF, b, ch, NBF:NB],
                    )
                cin2 = work.tile([112, 56], BF16, tag="cin2")
                for ch in range(2):
                    nc.sync.dma_start(
                        out=cin2[56 * ch : 56 * ch + 56, :],
                        in_=mdr[ch, :].rearrange("(h w) -> h w", w=56),
                    )
                pconv = psC.tile([128, 448], F32, tag="pconv")
                for dw in (3, 0, 1, 2, 4, 5, 6):
                    dws = dw - 3
                    wo0 = max(0, -dws)
                    wo1 = 56 - max(0, dws)
                    nc.tensor.matmul(
                        pconv[0:56, wo0:wo1],
                        lhsT=wband[:, dw, :],
                        rhs=cin2[:, wo0 + dws : wo1 + dws],
                        start=(dw == 3),
                        stop=(dw == 6),
                    )
                sawh = work.tile([56, 56], F32, tag="sawh")
                nc.scalar.activation(out=sawh, in_=pconv[0:56, 0:56], func=ACT.Sigmoid)
                sdr = dpool.tile([HW], F32, tag="sdr")
                nc.sync.dma_start(
                    out=sdr[:].rearrange("(h w) -> h w", w=56), in_=sawh
                )
                nc.sync.dma_start(
                    out=saf[0:P, b, 0:NBF],
                    in_=sdr[0:3072].rearrange("(p t) -> p t", t=NBF),
                )
                nc.sync.dma_start(
                    out=saf[0:HALF, b, NBF:NB],
                    in_=sdr[3072:HW].rearrange("(p o) -> p o", o=1),
                )
                nc.vector.tensor_copy(out=safb[:, b, :], in_=saf[:, b, :])

            # ------------- phase B3: XR *= sa, DMA out ----------------------
            def phase_b3(b):
                nc.vector.tensor_mul(
                    out=XR[:, b, 0:NDVE_B3, :],
                    in0=XR[:, b, 0:NDVE_B3, :],
                    in1=safb[:, b, 0:NDVE_B3][:, :, None].broadcast_to(
                        [P, NDVE_B3, C]
                    ),
                )
                for t in range(NDVE_B3, NB):
                    pp = P if t < NBF else HALF
                    nc.scalar.activation(
                        out=XR[0:pp, b, t, :],
                        in_=XR[0:pp, b, t, :],
                        func=ACT.Copy,
                        scale=saf[0:pp, b, t : t + 1],
                    )
                ring = nc.scalar if b % 2 == 0 else nc.sync
                ring.dma_start(
                    out=out_hwc[b, 0:3072, :].rearrange("(p t) c -> p t c", t=NBF),
                    in_=XR[0:P, b, 0:NBF, :],
                )
                ring.dma_start(
                    out=out_hwc[b, 3072:HW, :].rearrange("(p o) c -> p o c", o=1),
                    in_=XR[0:HALF, b, NBF:NB, :],
                )

            # ---------------- issue order ----------------------------------
            for b in range(NIMG):
                load(b)
            build_bands()
            cast(0)
            cast(1)
            cast(2)
            cast(3)
            phase_a(0)
            phase_a(1)
            phase_a(2)
            phase_a(3)
            phase_b1(0)
            phase_b2(0)
            phase_b1(1)
            phase_b2(1)
            phase_b3(0)
            phase_b1(2)
            phase_b2(2)
            phase_b3(1)
            phase_b1(3)
            phase_b2(3)
            phase_b3(2)
            phase_b3(3)

    nc.finalize()
    return nc


LAST_RESULTS = None


def kernel(x, w1, b1, w2, b2, conv_w):
    global LAST_RESULTS
    nc = _CACHE.get("nc")
    if nc is None:
        nc = _build_nc()
        _CACHE["nc"] = nc

    x = np.ascontiguousarray(np.asarray(x, dtype=np.float32))
    shards = np.split(x, NCORES, axis=0)
    common = {
        "w1": np.ascontiguousarray(np.asarray(w1, dtype=np.float32)),
        "b1": np.ascontiguousarray(np.asarray(b1, dtype=np.float32)),
        "w2": np.ascontiguousarray(np.asarray(w2, dtype=np.float32)),
        "b2": np.ascontiguousarray(np.asarray(b2, dtype=np.float32)),
        "conv_w": np.ascontiguousarray(np.asarray(conv_w, dtype=np.float32)),
    }
    in_maps = [dict(common, x=np.ascontiguousarray(s)) for s in shards]

    res = run_bass_kernel_spmd(
        nc,
        in_maps,
        core_ids=list(range(NCORES)),
        trace=bool(int(os.environ.get("CBAM_TRACE", "0"))),
    )
    LAST_RESULTS = res
    return np.concatenate(
        [np.asarray(r["out"]).astype(np.float32) for r in res.results], axis=0
    )


# revision 25
# speedup vs baseline: 1.0336x; 1.0336x over previous
"""CBAM block (channel + spatial attention) Trainium2 Bass kernel, v3.

Problem: x [32, 56, 56, 256] f32; data-parallel over batch across 8 NeuronCores
(4 images per core).  Everything is hardcoded for these shapes.

Per-core dataflow (B=4 images, each [3136(hw), 256(c)]), bf16 on chip:

  Layout: flat row n of an image maps to (partition p, block t) as
    group A: p in [0, 64),   t in [0, 25): n = 25*p + t
    group B: p in [64, 128), t in [0, 24): n = 1600 + 24*(p-64) + t

  Input: f32 on the two HWDGE rings (sync=group A, scalar=group B), then one
  big 3D ACT copy per image casts to bf16 (Xb); every later DVE scan runs at
  the 2x_1P bf16 rate.

  Phase A (channel attention): pairwise max tree over t (6 DVE ops) -> 2 PE
  transposes -> DVE max -> statsT col 1; sum over hw via 13 pair-accumulated
  PE matmuls (rhs [128, 512] bf16) -> psum row -> fold + transposes ->
  statsT col 0; tiny f32 MLP; ca broadcast by ones-matmul -> bca bf16.

  Phase B1: XR = Xb * bca as ONE 3D DVE op (in1 broadcast over t via
  stride-0 AP, still 2x).  sum_c / max_c as bf16 pairwise trees to width 16
  plus one small reduce -> maps bf16.

  Phase B2 (spatial 7x7 conv): Toeplitz bands built ONCE from an inline 0/1
  diagonal-mask constant: tmp98[(ch,dw,dh), (a,b)] = dmask * w98 (one DVE
  tensor_scalar), 7 selector matmuls fold dh -> tband [(ch,dw), (a,b)],
  DRAM round trip re-lays it as wband112 [(ch,h_in)=112, dw, h_out].
  Per image: maps -> flat DRAM -> cin2 [(ch,h_in), w] (6 small gpsimd DMAs),
  7 accumulated matmuls (lhsT = wband112[:, dw, :], w-shift via column
  windows) -> psum [56, 56], ACT sigmoid evac, 2 SBUF->SBUF DMAs -> saf.

  Phase B3: XR *= saf[p, t]: a few blocks on DVE tensor_scalar, the rest on
  ACT activation-with-scale; 2 bf16 out-DMAs; host casts to f32.
"""

import os

import numpy as np
import ml_dtypes

import concourse.bass as bass
import concourse.bacc as bacc
import concourse.tile as tile
from concourse import mybir
from concourse.bass_types import AP
from concourse.bass_utils import run_bass_kernel_spmd

F32 = mybir.dt.float32
BF16 = mybir.dt.bfloat16
AX = mybir.AxisListType
OP = mybir.AluOpType
ACT = mybir.ActivationFunctionType

P = 128          # partitions
NB = 25          # blocks in group A (group B has 24)
NBF = 24         # full-width blocks
HALF = 64        # partitions in group A / valid rows in block 24
C = 256          # channels
HW = 3136        # 56*56
GA = 1600        # rows in group A (64 * 25)
NIMG = 4         # images per core
NCORES = 8
NDVE_B3 = 14      # leading blocks of B3 applied on DVE; rest on ACT

_CACHE: dict = {}


def _build_nc() -> bass.Bass:
    nc = bacc.Bacc()

    x_d = nc.dram_tensor("x", [NIMG, 56, 56, C], F32, kind="ExternalInput")
    w1_d = nc.dram_tensor("w1", [C, 16], F32, kind="ExternalInput")
    b1_d = nc.dram_tensor("b1", [16], F32, kind="ExternalInput")
    w2_d = nc.dram_tensor("w2", [16, C], F32, kind="ExternalInput")
    b2_d = nc.dram_tensor("b2", [C], F32, kind="ExternalInput")
    cw_d = nc.dram_tensor("conv_w", [7, 7, 2, 1], F32, kind="ExternalInput")
    out_d = nc.dram_tensor("out", [NIMG, 56, 56, C], BF16, kind="ExternalOutput")

    ident_d = nc.inline_tensor(np.eye(128, dtype=np.float32), name="ident128")

    # dmask98[(ch,dh,dw), (a, b)] = 1 iff a - b == dh - 3  (bands over h)
    dm = np.zeros((7, 56, 56), dtype=ml_dtypes.bfloat16)
    for dh in range(7):
        for a in range(56):
            b = a - (dh - 3)
            if 0 <= b < 56:
                dm[dh, a, b] = 1.0
    dmask98_np = np.broadcast_to(
        dm[None, :, None, :, :], (2, 7, 7, 56, 56)
    ).reshape(98, HW)
    dmask98_d = nc.inline_tensor(np.ascontiguousarray(dmask98_np), name="dmask98")

    # sel98[(ch,dh,dw), (ch',dw')] = 1 iff (ch,dw) == (ch',dw')
    sel_np = np.zeros((98, 14), dtype=ml_dtypes.bfloat16)
    for ch in range(2):
        for dh in range(7):
            for dw in range(7):
                sel_np[ch * 49 + dh * 7 + dw, ch * 7 + dw] = 1.0
    sel98_d = nc.inline_tensor(sel_np, name="sel98")

    wscale_np = np.ones((98, 1), dtype=np.float32)
    wscale_np[0:49] = 1.0 / C  # fold the mean 1/C into the ch0 conv taps
    wscale_d = nc.inline_tensor(wscale_np, name="wscale")

    x_hwc = x_d[:].rearrange("b h w c -> b (h w) c")
    out_hwc = out_d[:].rearrange("b h w c -> b (h w) c")

    with tile.TileContext(nc) as tc:
        import contextlib

        with contextlib.ExitStack() as ctx:
            cpool = ctx.enter_context(tc.tile_pool(name="cpool", bufs=1))
            xpool = ctx.enter_context(tc.tile_pool(name="xpool", bufs=1))
            work = ctx.enter_context(tc.tile_pool(name="work", bufs=2))
            small = ctx.enter_context(tc.tile_pool(name="small", bufs=3))
            psS = ctx.enter_context(tc.tile_pool(name="psS", bufs=2, space="PSUM"))
            psM = ctx.enter_context(tc.tile_pool(name="psM", bufs=3, space="PSUM"))
            psC = ctx.enter_context(tc.tile_pool(name="psC", bufs=2, space="PSUM"))
            dpool = ctx.enter_context(tc.tile_pool(name="dpool", bufs=2, space="DRAM"))

            # ---------------- constants & weights ----------------
            identf = cpool.tile([128, 128], F32)
            nc.gpsimd.dma_start(out=identf, in_=ident_d[:])
            identb = cpool.tile([128, 128], BF16)
            nc.vector.tensor_copy(out=identb, in_=identf)

            w1_sb = cpool.tile([128, 2, 16], F32)
            nc.gpsimd.dma_start(out=w1_sb, in_=w1_d[:].rearrange("(j p) m -> p j m", p=128))
            w2_sb = cpool.tile([16, 2, 128], F32)
            nc.gpsimd.dma_start(out=w2_sb, in_=w2_d[:].rearrange("k (j m) -> k j m", j=2))
            b1_sb = cpool.tile([16, 1], F32)
            nc.gpsimd.dma_start(out=b1_sb, in_=b1_d[:].rearrange("(p o) -> p o", o=1))
            b2_sb = cpool.tile([128, 2], F32)
            nc.gpsimd.dma_start(out=b2_sb, in_=b2_d[:].rearrange("(j p) -> p j", p=128))
            b2x2 = cpool.tile([128, 2], F32)

            # conv weights in (ch, dh, dw) partition order
            w98f = cpool.tile([98, 1], F32)
            nc.gpsimd.dma_start(out=w98f, in_=cw_d[:].transpose([2, 0, 1, 3]))

            ones_col = cpool.tile([128, 1], BF16)
            nc.vector.memset(ones_col, 1.0)
            ones_rb = cpool.tile([1, 128], BF16)
            nc.vector.memset(ones_rb, 1.0)

            # ---------------- one-time Toeplitz band build ------------------
            # (issued AFTER the input loads so its ACT evac copies don't sit
            # ahead of the group-B load DMAs in the scalar engine FIFO)
            sel_sb = cpool.tile([98, 14], BF16)
            nc.gpsimd.dma_start(out=sel_sb, in_=sel98_d[:])
            wband = cpool.tile([112, 7, 56], BF16)

            wscale = cpool.tile([98, 1], F32)
            nc.gpsimd.dma_start(out=wscale, in_=wscale_d[:])

            def build_bands():
                nc.scalar.activation(out=b2x2, in_=b2_sb, func=ACT.Copy, scale=2.0)
                nc.vector.tensor_mul(out=w98f, in0=w98f, in1=wscale)
                dmask_sb = work.tile([98, HW], BF16, tag="s1", bufs=1)
                nc.gpsimd.dma_start(out=dmask_sb, in_=dmask98_d[:])
                tmp98 = work.tile([98, HW], BF16, tag="m12", bufs=1)
                nc.vector.tensor_scalar_mul(out=tmp98, in0=dmask_sb, scalar1=w98f)

                tband = work.tile([14, HW], BF16, tag="s1x", bufs=1)
                for c7 in range(7):
                    ptb = psC.tile([128, 448], F32, tag="pconv", name=f"ptb{c7}")
                    nc.tensor.matmul(
                        ptb[0:14, :],
                        lhsT=sel_sb,
                        rhs=tmp98[:, 448 * c7 : 448 * (c7 + 1)],
                        start=True,
                        stop=True,
                    )
                    nc.scalar.copy(
                        out=tband[:, 448 * c7 : 448 * (c7 + 1)], in_=ptb[0:14, :]
                    )
                tbd = dpool.tile([14, HW], BF16, tag="tbd", bufs=1)
                nc.gpsimd.dma_start(out=tbd[:], in_=tband)
                tb = tbd[:]
                for ch in range(2):
                    in_ap = AP(
                        tb.tensor,
                        tb.offset + ch * 7 * HW,
                        [[56, 56], [HW, 7], [1, 56]],
                    )
                    nc.gpsimd.dma_start(
                        out=wband[56 * ch : 56 * ch + 56, :, :], in_=in_ap
                    )

            # ---------------- big SBUF state ----------------
            Xb = xpool.tile([P, NIMG, NB, C], BF16)
            XR = Xb  # B1/B3 run in place; Xb is dead after phase A reads it
            xf_tiles = {}
            bca = cpool.tile([P, NIMG, C], BF16)
            maps = cpool.tile([P, NIMG, 2, NB], BF16)
            saf = cpool.tile([P, NIMG, NB], F32)
            safb = cpool.tile([P, NIMG, NB], BF16)
            statsT = cpool.tile([128, NIMG, 2, 2], F32)
            caT = cpool.tile([128, NIMG, 2], F32)

            # PE warm-up matmuls touching constant lhsT sources
            pwu = psM.tile([128, 4], F32, tag="mlp")
            nc.tensor.matmul(
                pwu[0:4, 0:4], lhsT=identb[:, 0:4], rhs=identb[:, 0:4],
                start=True, stop=True,
            )
            nc.tensor.matmul(
                pwu[0:4, 0:4], lhsT=identf[:, 0:4], rhs=identf[:, 0:4],
                start=True, stop=True,
            )
            nc.tensor.matmul(
                pwu[0:1, 0:4], lhsT=ones_col[0:1, :], rhs=ones_rb[:, 0:4],
                start=True, stop=True,
            )
            nc.tensor.matmul(
                pwu[0:4, 0:4], lhsT=w1_sb[:, 0, 0:4], rhs=w1_sb[:, 0, 0:4],
                start=True, stop=True,
            )
            nc.tensor.matmul(
                pwu[0:4, 0:4], lhsT=w2_sb[:, 0, 0:4], rhs=w2_sb[:, 0, 0:4],
                start=True, stop=True,
            )
            nc.tensor.matmul(
                pwu[0:4, 0:4], lhsT=wband[0:4, 0, 0:4], rhs=wband[0:4, 0, 0:4],
                start=True, stop=True,
            )

            # ---------------- DMA in (f32, both HWDGE rings) ----------------
            def load(b):
                if b in (0, 3):
                    nc.gpsimd.dma_start(
                        out=Xb[0:P, b, 0:NBF, :],
                        in_=x_hwc[b, 0:3072, :].rearrange("(p t) c -> p t c", t=NBF),
                    )
                    nc.gpsimd.dma_start(
                        out=Xb[0:HALF, b, NBF:NB, :],
                        in_=x_hwc[b, 3072:HW, :].rearrange("(p o) c -> p o c", o=1),
                    )
                    return
                xf = work.tile([P, NB, C], F32, tag="xf", name=f"xf{b}", bufs=2)
                xf_tiles[b] = xf
                ring = nc.sync if b == 1 else nc.scalar
                ring.dma_start(
                    out=xf[0:P, 0:NBF, :],
                    in_=x_hwc[b, 0:3072, :].rearrange("(p t) c -> p t c", t=NBF),
                )
                ring.dma_start(
                    out=xf[0:HALF, NBF:NB, :],
                    in_=x_hwc[b, 3072:HW, :].rearrange("(p o) c -> p o c", o=1),
                )

            def cast(b):
                if b in (1, 2):
                    nc.scalar.copy(out=Xb[:, b], in_=xf_tiles[b][:])

            # ---------------- phase A (channel attention) -------------------
            def phase_a(b):
                m12 = work.tile([P, 12, C], BF16, tag="m12", bufs=1)
                m6 = work.tile([P, 6, C], BF16, tag="m6", bufs=1)
                m3 = work.tile([P, 3, C], BF16, tag="m3", bufs=1)
                acc = work.tile([P, C], BF16, tag="acc")
                nc.vector.tensor_max(
                    out=m12, in0=Xb[:, b, 0:12, :], in1=Xb[:, b, 12:NBF, :]
                )
                nc.vector.tensor_max(out=m6, in0=m12[:, 0:6, :], in1=m12[:, 6:12, :])
                nc.vector.tensor_max(out=m3, in0=m6[:, 0:3, :], in1=m6[:, 3:6, :])
                nc.vector.tensor_max(out=acc, in0=m3[:, 0, :], in1=m3[:, 1, :])
                nc.vector.tensor_max(out=acc, in0=acc, in1=m3[:, 2, :])
                nc.vector.tensor_max(
                    out=acc[0:HALF], in0=acc[0:HALF], in1=Xb[0:HALF, b, NBF, :]
                )
                pmaxT = psM.tile([128, 2, 128], BF16, tag="mlp")
                for j in range(2):
                    nc.tensor.transpose(
                        pmaxT[:, j, :], acc[:, j * 128 : (j + 1) * 128], identb
                    )
                nc.vector.tensor_reduce(
                    out=statsT[:, b, :, 1:2], in_=pmaxT, axis=AX.X, op=OP.max
                )

                # sum over hw: 13 pair-accumulated matmuls -> [1, 512] psum
                ps = psS.tile([1, 512], F32, tag="ps")
                for k in range(12):
                    nc.tensor.matmul(
                        ps,
                        lhsT=ones_col,
                        rhs=Xb[:, b, 2 * k : 2 * k + 2, :],
                        start=(k == 0),
                        stop=False,
                    )
                nc.tensor.matmul(
                    ps[:, 0:C],
                    lhsT=ones_col[0:HALF],
                    rhs=Xb[0:HALF, b, NBF, :],
                    start=False,
                    stop=True,
                )
                ssb = small.tile([1, 512], F32, tag="ssb")
                nc.vector.tensor_copy(out=ssb, in_=ps)
                srow = small.tile([1, C], F32, tag="srow")
                nc.vector.tensor_add(out=srow, in0=ssb[:, 0:C], in1=ssb[:, C:512])
                pavgT = psM.tile([128, 2, 1], F32, tag="mlp")
                for j in range(2):
                    nc.tensor.transpose(
                        pavgT[:, j, :],
                        srow[:, j * 128 : (j + 1) * 128],
                        identf[0:1, 0:1],
                    )
                nc.scalar.activation(
                    out=statsT[:, b, :, 0:1], in_=pavgT, func=ACT.Copy, scale=1.0 / HW
                )

                # MLP layer 1: h = relu(W1^T statsT + b1)
                ph = psM.tile([16, 2], F32, tag="mlp")
                for j in range(2):
                    nc.tensor.matmul(
                        ph,
                        lhsT=w1_sb[:, j, :],
                        rhs=statsT[:, b, j, :],
                        start=(j == 0),
                        stop=(j == 1),
                    )
                h_sb = small.tile([16, 2], F32, tag="h_sb")
                nc.scalar.activation(
                    out=h_sb, in_=ph, func=ACT.Relu, bias=b1_sb, scale=1.0
                )

                # layer 2 + combine + sigmoid -> caT [128, 2]
                for j in range(2):
                    pc = psM.tile([128, 2], F32, tag="mlp")
                    nc.tensor.matmul(
                        pc, lhsT=w2_sb[:, j, :], rhs=h_sb, start=True, stop=True
                    )
                    pc_sb = small.tile([128, 2], F32, tag="pc_sb")
                    nc.vector.tensor_copy(out=pc_sb, in_=pc)
                    catmp = small.tile([128, 1], F32, tag="catmp")
                    nc.vector.tensor_add(
                        out=catmp, in0=pc_sb[:, 0:1], in1=pc_sb[:, 1:2]
                    )
                    nc.scalar.activation(
                        out=caT[:, b, j : j + 1],
                        in_=catmp,
                        func=ACT.Sigmoid,
                        bias=b2x2[:, j : j + 1],
                        scale=1.0,
                    )

                # broadcast ca over partitions: bca[:, b, :] (bf16)
                pcr = psM.tile([1, 2, 128], F32, tag="mlp")
                for j in range(2):
                    nc.tensor.transpose(pcr[:, j, :], caT[:, b, j : j + 1], identf)
                ca_row = small.tile([1, C], BF16, tag="ca_row")
                nc.vector.tensor_copy(
                    out=ca_row, in_=pcr.rearrange("p j m -> p (j m)")
                )
                pbca = psS.tile([P, C], F32, tag="ps")
                nc.tensor.matmul(pbca, lhsT=ones_rb, rhs=ca_row, start=True, stop=True)
                nc.scalar.copy(out=bca[:, b, :], in_=pbca)

            # ------------- phase B1: XR = Xb * ca; spatial maps -------------
            def phase_b1(b):
                nc.vector.tensor_mul(
                    out=XR[:, b],
                    in0=Xb[:, b],
                    in1=bca[:, b, :][:, None, :].broadcast_to([P, NB, C]),
                )
                s1 = work.tile([P, NB, 128], BF16, tag="s1", bufs=1)
                s2 = work.tile([P, NB, 64], BF16, tag="s2", bufs=1)
                s3 = work.tile([P, NB, 32], BF16, tag="s3", bufs=1)
                s4 = work.tile([P, NB, 16], BF16, tag="s4", bufs=1)
                nc.vector.tensor_add(
                    out=s1, in0=XR[:, b, :, 0:128], in1=XR[:, b, :, 128:C]
                )
                nc.vector.tensor_add(out=s2, in0=s1[:, :, 0:64], in1=s1[:, :, 64:128])
                nc.vector.tensor_add(out=s3, in0=s2[:, :, 0:32], in1=s2[:, :, 32:64])
                nc.vector.tensor_add(out=s4, in0=s3[:, :, 0:16], in1=s3[:, :, 16:32])
                with nc.allow_low_precision("mean map tolerates bf16"):
                    nc.vector.tensor_reduce(
                        out=maps[:, b, 0, :], in_=s4, axis=AX.X, op=OP.add
                    )
                nc.vector.tensor_max(
                    out=s1, in0=XR[:, b, :, 0:128], in1=XR[:, b, :, 128:C]
                )
                nc.vector.tensor_max(out=s2, in0=s1[:, :, 0:64], in1=s1[:, :, 64:128])
                nc.vector.tensor_max(out=s3, in0=s2[:, :, 0:32], in1=s2[:, :, 32:64])
                nc.vector.tensor_max(out=s4, in0=s3[:, :, 0:16], in1=s3[:, :, 16:32])
                nc.vector.tensor_reduce(
                    out=maps[:, b, 1, :], in_=s4, axis=AX.X, op=OP.max
                )

            # ------------- phase B2: 7x7 conv via Toeplitz bands ------------
            def phase_b2(b):
                mdr = dpool.tile([2, HW], BF16, tag="mdr")
                for ch in range(2):
                    nc.gpsimd.dma_start(
                        out=mdr[ch, 0:3072].rearrange("(p t) -> p t", t=NBF),
                        in_=maps[0:P, b, ch, 0:NBF],
                    )
                    nc.gpsimd.dma_start(
                        out=mdr[ch, 3072:HW].rearrange("(p o) -> p o", o=1),
                        in_=maps[0:HALF, b, ch, NBF:NB],
                    )
                cin2 = work.tile([112, 56], BF16, tag="cin2")
                for ch in range(2):
                    nc.sync.dma_start(
                        out=cin2[56 * ch : 56 * ch + 56, :],
                        in_=mdr[ch, :].rearrange("(h w) -> h w", w=56),
                    )
                pconv = psC.tile([128, 448], F32, tag="pconv")
                for dw in (3, 0, 1, 2, 4, 5, 6):
                    dws = dw - 3
                    wo0 = max(0, -dws)
                    wo1 = 56 - max(0, dws)
                    nc.tensor.matmul(
                        pconv[0:56, wo0:wo1],
                        lhsT=wband[:, dw, :],
                        rhs=cin2[:, wo0 + dws : wo1 + dws],
                        start=(dw == 3),
                        stop=(dw == 6),
                    )
                sawh = work.tile([56, 56], F32, tag="sawh")
                nc.scalar.activation(out=sawh, in_=pconv[0:56, 0:56], func=ACT.Sigmoid)
                sdr = dpool.tile([HW], F32, tag="sdr")
                nc.sync.dma_start(
                    out=sdr[:].rearrange("(h w) -> h w", w=56), in_=sawh
                )
                nc.sync.dma_start(
                    out=saf[0:P, b, 0:NBF],
                    in_=sdr[0:3072].rearrange("(p t) -> p t", t=NBF),
                )
                nc.sync.dma_start(
                    out=saf[0:HALF, b, NBF:NB],
                    in_=sdr[3072:HW].rearrange("(p o) -> p o", o=1),
                )
                nc.vector.tensor_copy(out=safb[:, b, :], in_=saf[:, b, :])

            # ------------- phase B3: XR *= sa, DMA out ----------------------
            def phase_b3(b):
                nc.vector.tensor_mul(
                    out=XR[:, b, 0:NDVE_B3, :],
                    in0=XR[:, b, 0:NDVE_B3, :],
                    in1=safb[:, b, 0:NDVE_B3][:, :, None].broadcast_to(
                        [P, NDVE_B3, C]
                    ),
                )
                for t in range(NDVE_B3, NB):
                    pp = P if t < NBF else HALF
                    nc.scalar.activation(
                        out=XR[0:pp, b, t, :],
                        in_=XR[0:pp, b, t, :],
                        func=ACT.Copy,
                        scale=saf[0:pp, b, t : t + 1],
                    )
                ring = nc.scalar if b % 2 == 0 else nc.sync
                ring.dma_start(
                    out=out_hwc[b, 0:3072, :].rearrange("(p t) c -> p t c", t=NBF),
                    in_=XR[0:P, b, 0:NBF, :],
                )
                ring.dma_start(
                    out=out_hwc[b, 3072:HW, :].rearrange("(p o) c -> p o c", o=1),
                    in_=XR[0:HALF, b, NBF:NB, :],
                )

            # ---------------- issue order ----------------------------------
            for b in range(NIMG):
                load(b)
            build_bands()
            cast(0)
            cast(1)
            cast(2)
            cast(3)
            phase_a(0)
            phase_a(1)
            phase_a(2)
            phase_a(3)
            phase_b1(0)
            phase_b2(0)
            phase_b1(1)
            phase_b2(1)
            phase_b3(0)
            phase_b1(2)
            phase_b2(2)
            phase_b3(1)
            phase_b1(3)
            phase_b2(3)
            phase_b3(2)
            phase_b3(3)

    nc.finalize()
    return nc


LAST_RESULTS = None


def kernel(x, w1, b1, w2, b2, conv_w):
    global LAST_RESULTS
    nc = _CACHE.get("nc")
    if nc is None:
        nc = _build_nc()
        _CACHE["nc"] = nc

    x = np.ascontiguousarray(np.asarray(x, dtype=np.float32))
    shards = np.split(x, NCORES, axis=0)
    common = {
        "w1": np.ascontiguousarray(np.asarray(w1, dtype=np.float32)),
        "b1": np.ascontiguousarray(np.asarray(b1, dtype=np.float32)),
        "w2": np.ascontiguousarray(np.asarray(w2, dtype=np.float32)),
        "b2": np.ascontiguousarray(np.asarray(b2, dtype=np.float32)),
        "conv_w": np.ascontiguousarray(np.asarray(conv_w, dtype=np.float32)),
    }
    in_maps = [dict(common, x=np.ascontiguousarray(s)) for s in shards]

    res = run_bass_kernel_spmd(
        nc,
        in_maps,
        core_ids=list(range(NCORES)),
        trace=bool(int(os.environ.get("CBAM_TRACE", "0"))),
    )
    LAST_RESULTS = res
    return np.concatenate(
        [np.asarray(r["out"]).astype(np.float32) for r in res.results], axis=0
    )
